# revision 21
# baseline (speedup 1.0000x reference)
"""Trainium2 Bass kernel for the EnhancedBalSCL contrastive loss.

Full inputs in, full (scalar) output out. Internally data-parallel over the
batch dim across 8 NeuronCores; each core owns 512 rows of the batch and
produces a partial sum of per-sample losses; the host sums the 8 partials.

Math reformulation (validated to ~1e-6 vs the jax reference):
  w[k] = 1/(counts[t_k]+1), v[j] = 1/(counts[j]+1)
  denom[i] = sum_k exp(10*raw[i,k]) * w[k] + sum_j exp(10*rawc[i,j]) * v[j]
  H[:,j]   = sum_{k: t_k=j} F[k,:]           (class-summed features, host)
  U[:,i]   = (H+C).T[:, t_i]                 (host gather, per-core slice)
  P[i]     = sum_d F[i,d] * U[d,i]           (same-class raw sum + center raw)
  per_sample[i] = log(denom[i]) - (P[i] - ||F_i||^2) * 10 / counts[t_i]
  loss = mean(per_sample)
where raw = F F^T (no tau), rawc = F C^T.  The eps terms of the reference are
negligible at these scales (validated numerically).

Device mapping per core (512 rows = 4 row-tiles of 128):
  PE  : raw blocks [128,1024] (bf16, fp32 accum), rawc blocks, P via diagonal
        128x128 blocks of F_loc @ U, partition-sum via ones matmul.
  ACT : exp(10*x) in place in PSUM; final log.
  DVE : scalar_tensor_tensor fused (exp * w) row-sum from PSUM, diag extract
        via identity mask, per-sample assembly.
"""

import numpy as np
import ml_dtypes

_B, _D, _C, _M = 4096, 1024, 1000, 8
_BL = _B // _M            # 512 rows per core
_RT = _BL // 128          # 4 row tiles per core
_KT = _D // 128           # 8 contraction tiles
_NBW = 1024               # big-matmul column block width
_NB = _B // _NBW          # 4 column blocks
_SCALE = 10.0             # 1/tau

_CACHE = {}


def _build_nc():
    import concourse.bass as bass
    import concourse.mybir as mybir
    from concourse import bacc, tile
    from contextlib import ExitStack

    f32 = mybir.dt.float32
    bf16 = mybir.dt.bfloat16
    AF = mybir.ActivationFunctionType
    OP = mybir.AluOpType
    AX = mybir.AxisListType

    nc = bacc.Bacc("TRN2", target_bir_lowering=False, debug=False,
                   num_devices=_M)
    ft_d = nc.declare_dram_parameter("ft", [_NB, _KT, 128, _NBW], bf16, isOutput=False)
    fl_d = nc.declare_dram_parameter("ftloc", [_KT, 128, _BL], bf16, isOutput=False)
    rc_d = nc.declare_dram_parameter("rc", [_KT, 128, _C], bf16, isOutput=False)
    u_d = nc.declare_dram_parameter("u", [_KT, 128, _BL], bf16, isOutput=False)
    w_d = nc.declare_dram_parameter("wrow", [1, _B], f32, isOutput=False)
    v_d = nc.declare_dram_parameter("vrow", [1, _C], f32, isOutput=False)
    dg_d = nc.declare_dram_parameter("diagc", [128, _RT], f32, isOutput=False)
    rn_d = nc.declare_dram_parameter("rnp", [128, _RT], f32, isOutput=False)
    id_d = nc.declare_dram_parameter("ident", [128, 128], f32, isOutput=False)
    on_d = nc.declare_dram_parameter("ones", [128, 1], f32, isOutput=False)
    out_d = nc.declare_dram_parameter("out", [1, 1], f32, isOutput=True)

    with tile.TileContext(nc) as tc, ExitStack() as ctx:
        consts = ctx.enter_context(tc.tile_pool(name="consts", bufs=1))
        psum = ctx.enter_context(tc.tile_pool(name="psum", bufs=1, space="PSUM"))
        sm = ctx.enter_context(tc.tile_pool(name="sm", bufs=8))

        # --- persistent SBUF residents -------------------------------------
        # HWDGE (sync) queue order = urgency: w/v rows (gate the first STTs
        # via the partition broadcasts), then lhsT, then the per-n ft merged
        # chunks.  rc/u/ident go through gpsimd (SWDGE) — a parallel DGE
        # path — they are only needed later.  Merged (one DMA per logical
        # tensor) to amortize the ~0.6us/DMA HWDGE setup.
        wrow = consts.tile([1, _B], f32, tag="wrow")
        nc.sync.dma_start(wrow[:], w_d[:])
        vrow = consts.tile([1, _C], f32, tag="vrow")
        nc.sync.dma_start(vrow[:], v_d[:])
        wb = consts.tile([128, _B], f32, tag="wb")
        nc.gpsimd.partition_broadcast(wb[:], wrow[:])
        vb = consts.tile([128, _C], f32, tag="vb")
        nc.gpsimd.partition_broadcast(vb[:], vrow[:])

        # fl and the first ft block k-granular so the first matmuls can start
        # as soon as their own k-chunk lands; later ft blocks merged (one DMA
        # each) to amortize HWDGE setup.
        fl = consts.tile([128, _KT * _BL], bf16, tag="fl")
        for k in range(_KT):
            nc.sync.dma_start(fl[:, k * _BL:(k + 1) * _BL], fl_d[k])
        ftt = []
        for n in range(_NB):
            t = consts.tile([128, _KT * _NBW], bf16, tag=f"ft{n}")
            if n == 0:
                for k in range(_KT):
                    nc.sync.dma_start(t[:, k * _NBW:(k + 1) * _NBW], ft_d[n, k])
            else:
                nc.sync.dma_start(t[:].rearrange("p (k c) -> p k c", k=_KT),
                                  ft_d[n].rearrange("k p c -> p k c"))
            ftt.append(t)

        rct = consts.tile([128, _KT * _C], bf16, tag="rct")
        nc.gpsimd.dma_start(rct[:].rearrange("p (k c) -> p k c", k=_KT),
                            rc_d[:].rearrange("k p c -> p k c"))
        ut = consts.tile([128, _KT * _BL], bf16, tag="ut")
        nc.gpsimd.dma_start(ut[:].rearrange("p (k c) -> p k c", k=_KT),
                            u_d[:].rearrange("k p c -> p k c"))
        ident = consts.tile([128, 128], f32, tag="ident")
        nc.gpsimd.dma_start(ident[:], id_d[:])
        ones = consts.tile([128, 1], f32, tag="ones")
        nc.gpsimd.dma_start(ones[:], on_d[:])
        dgc = consts.tile([128, _RT], f32, tag="dgc")
        nc.gpsimd.dma_start(dgc[:], dg_d[:])
        rnp = consts.tile([128, _RT], f32, tag="rnp")
        nc.gpsimd.dma_start(rnp[:], rn_d[:])

        # slice helpers
        lhs = [[fl[:, k * _BL + m * 128: k * _BL + (m + 1) * 128]
                for k in range(_KT)] for m in range(_RT)]
        accs = [consts.tile([128, 5], f32, tag=f"acc{m}", name=f"acc{m}")
                for m in range(_RT)]
        pstile = consts.tile([128, _RT], f32, tag="pstile")

        def big_block(n, m):
            ps = psum.tile([128, _NBW], f32, tag="big", bufs=3, name="psb")
            for k in range(_KT):
                rk = ftt[n][:, k * _NBW: (k + 1) * _NBW]
                nc.tensor.matmul(ps[:, 0:512], lhs[m][k], rk[:, 0:512],
                                 start=(k == 0), stop=(k == _KT - 1))
                nc.tensor.matmul(ps[:, 512:1024], lhs[m][k], rk[:, 512:1024],
                                 start=(k == 0), stop=(k == _KT - 1))
            nc.scalar.activation(ps[:], ps[:], AF.Exp, scale=_SCALE)
            nc.vector.scalar_tensor_tensor(
                out=ps[:], in0=ps[:], scalar=1.0,
                in1=wb[:, n * _NBW:(n + 1) * _NBW],
                op0=OP.mult, op1=OP.mult,
                accum_out=accs[m][:, n:n + 1])

        def centers_block(m):
            ps = psum.tile([128, _NBW], f32, tag="big", bufs=3, name="psc")
            for k in range(_KT):
                rk = rct[:, k * _C: (k + 1) * _C]
                nc.tensor.matmul(ps[:, 0:512], lhs[m][k], rk[:, 0:512],
                                 start=(k == 0), stop=(k == _KT - 1))
                nc.tensor.matmul(ps[:, 512:_C], lhs[m][k], rk[:, 512:_C],
                                 start=(k == 0), stop=(k == _KT - 1))
            nc.scalar.activation(ps[:, :_C], ps[:, :_C], AF.Exp, scale=_SCALE)
            nc.vector.scalar_tensor_tensor(
                out=ps[:, :_C], in0=ps[:, :_C], scalar=1.0, in1=vb[:],
                op0=OP.mult, op1=OP.mult,
                accum_out=accs[m][:, 4:5])

        p4 = consts.tile([128, _RT], f32, tag="p4")
        denom4 = consts.tile([128, _RT], f32, tag="denom4")

        def udiag_block(m):
            ps = psum.tile([128, _NBW], f32, tag="big", bufs=3, name="psu")
            for k in range(_KT):
                uk = ut[:, k * _BL + m * 128: k * _BL + (m + 1) * 128]
                nc.tensor.matmul(ps[:, :128], lhs[m][k], uk,
                                 start=(k == 0), stop=(k == _KT - 1))
            nc.vector.scalar_tensor_tensor(
                out=ps[:, :128], in0=ps[:, :128], scalar=1.0, in1=ident[:],
                op0=OP.mult, op1=OP.mult,
                accum_out=p4[:, m:m + 1])

        def finals():
            # all four row tiles at once in [128, 4]-wide ops
            for m in range(_RT):
                nc.vector.tensor_reduce(denom4[:, m:m + 1], accs[m][:, 0:5],
                                        axis=AX.X, op=OP.add)
            logd = sm.tile([128, _RT], f32, tag="logd", name="logd")
            # denom is O(1e3); the reference's +1e-8 is far below fp32 ulp
            nc.scalar.activation(logd[:], denom4[:], AF.Ln)
            t1 = sm.tile([128, _RT], f32, tag="t1", name="t1")
            nc.vector.tensor_tensor(out=t1[:], in0=p4[:], in1=dgc[:], op=OP.subtract)
            nc.vector.tensor_tensor(out=t1[:], in0=t1[:], in1=rnp[:], op=OP.mult)
            nc.vector.tensor_tensor(out=pstile[:], in0=logd[:], in1=t1[:],
                                    op=OP.subtract)

        # --- main schedule --------------------------------------------------
        # n-outer over the big blocks keeps each 2MB ft chunk feeding 4 row
        # tiles of PE work; centers/udiag interleave mid-stream (their SWDGE
        # DMAs land early), so per-m finals can run before the last big block
        # finishes and the final partition-sum matmul has no tail stall.
        for m in range(_RT):
            big_block(0, m)
        for m in range(_RT):
            big_block(1, m)
        for m in range(_RT):
            centers_block(m)
            udiag_block(m)
        for m in range(_RT):
            big_block(2, m)
        for m in range(_RT):
            big_block(3, m)
        finals()

        # partition sum -> scalar partial (ones matmul reduces partitions)
        ps = psum.tile([128, _NBW], f32, tag="big", bufs=3, name="psf")
        nc.tensor.matmul(ps[:1, :_RT], ones[:], pstile[:], start=True, stop=True)
        final = consts.tile([1, 1], f32, tag="final")
        nc.vector.tensor_reduce(final[:], ps[:1, :_RT], axis=AX.X, op=OP.add)
        nc.sync.dma_start(out_d[:], final[:])

    nc.compile()
    return nc


def _get_nc():
    if "nc" not in _CACHE:
        _CACHE["nc"] = _build_nc()
    return _CACHE["nc"]


def _prep_inputs(centers, features, targets):
    bf16 = ml_dtypes.bfloat16
    F = np.ascontiguousarray(features, dtype=np.float32)      # [B, D]
    Cen = np.ascontiguousarray(centers, dtype=np.float32)     # [C, D]
    t = np.asarray(targets).astype(np.int64).ravel()          # [B]

    counts = np.bincount(t, minlength=_C).astype(np.float32)  # [C]
    w = (1.0 / (counts[t] + 1.0)).astype(np.float32)          # [B]
    v = (1.0 / (counts + 1.0)).astype(np.float32)             # [C]
    H = np.zeros((_C, _D), dtype=np.float32)
    np.add.at(H, t, F)                                        # class sums
    R2 = H + Cen                                              # [C, D]

    Fb = F.astype(bf16)                                       # bf16 features
    FT = np.ascontiguousarray(Fb.T)                           # [D, B] bf16
    ft = np.ascontiguousarray(
        FT.reshape(_KT, 128, _NB, _NBW).transpose(2, 0, 1, 3))
    rc = np.ascontiguousarray(Cen.astype(bf16).T).reshape(_KT, 128, _C)
    U_all = R2.astype(bf16).T[:, t]                           # [D, B] gathered

    diag = (Fb.astype(np.float32) ** 2).sum(axis=1)           # matches device mm
    rnp = (np.float32(_SCALE) / counts[t]).astype(np.float32)

    wrow = w.reshape(1, _B)
    vrow = v.reshape(1, _C)
    ident = np.eye(128, dtype=np.float32)
    ones = np.ones((128, 1), dtype=np.float32)

    def col(x_loc):  # [512] -> [128, RT] with (p, m) = x[m*128+p]
        return np.ascontiguousarray(x_loc.reshape(_RT, 128).T)

    in_maps = []
    for c in range(_M):
        R = c * _BL
        ftloc = np.ascontiguousarray(FT[:, R:R + _BL]).reshape(_KT, 128, _BL)
        uloc = np.ascontiguousarray(U_all[:, R:R + _BL]).reshape(_KT, 128, _BL)
        in_maps.append({
            "ft": ft, "ftloc": ftloc, "rc": rc, "u": uloc,
            "wrow": wrow, "vrow": vrow,
            "diagc": col(diag[R:R + _BL]),
            "rnp": col(rnp[R:R + _BL]),
            "ident": ident, "ones": ones,
        })
    return in_maps


def _run(inputs, trace=False, **trace_kwargs):
    from concourse.bass_utils import run_bass_kernel_spmd
    nc = _get_nc()
    in_maps = _prep_inputs(**inputs)
    res = run_bass_kernel_spmd(nc, in_maps, core_ids=list(range(_M)),
                               trace=trace, **trace_kwargs)
    total = sum(float(r["out"][0, 0]) for r in res.results)
    return np.float32(total / _B), res


def kernel(centers, features, targets):
    out, _ = _run({"centers": centers, "features": features, "targets": targets})
    return out


# revision 25
# speedup vs baseline: 20598.7585x; 20598.7585x over previous
"""Trainium2 Bass kernel for the EnhancedBalSCL contrastive loss.

Full inputs in, full (scalar) output out. Internally data-parallel over the
batch dim across 8 NeuronCores; each core owns 512 rows of the batch and
produces a partial sum of per-sample losses; the host sums the 8 partials.

Math reformulation (validated to ~1e-6 vs the jax reference):
  w[k] = 1/(counts[t_k]+1), v[j] = 1/(counts[j]+1)
  denom[i] = sum_k exp(10*raw[i,k]) * w[k] + sum_j exp(10*rawc[i,j]) * v[j]
  H[:,j]   = sum_{k: t_k=j} F[k,:]           (class-summed features, host)
  U[:,i]   = (H+C).T[:, t_i]                 (host gather, per-core slice)
  P[i]     = sum_d F[i,d] * U[d,i]           (same-class raw sum + center raw)
  per_sample[i] = log(denom[i]) - (P[i] - ||F_i||^2) * 10 / counts[t_i]
  loss = mean(per_sample)
where raw = F F^T (no tau), rawc = F C^T.  The eps terms of the reference are
negligible at these scales (validated numerically).

Precision: the dominant F F^T matmul runs in fp8 e4m3 with DoubleRow (2 fp8
MACs/cell/cycle); its only systematic error — the fp8-squared diagonal inside
the denominator — is corrected exactly with a host-computed per-sample additive
term, leaving rel err ~7e-6 (validated on host).  Everything else is bf16
operands with fp32 accumulation.

Device mapping per core (512 rows = 4 row-tiles of 128):
  PE  : raw blocks [128,1024] (fp8 DoubleRow, 4 super-K tiles of 256),
        rawc blocks (bf16), P via diagonal 128x128 blocks of F_loc @ U,
        partition-sum via ones matmul.
  ACT : exp(10*x) in place in PSUM; final log.
  DVE : scalar_tensor_tensor fused (exp * w) row-sum from PSUM, diag extract
        via identity mask, per-sample assembly.
"""

import numpy as np
import ml_dtypes

_B, _D, _C, _M = 4096, 1024, 1000, 8
_BL = _B // _M            # 512 rows per core
_RT = _BL // 128          # 4 row tiles per core
_KT = _D // 128           # 8 contraction tiles (bf16 path)
_JT = _D // 256           # 4 super-K tiles (fp8 DoubleRow path)
_NBW = 1024               # big-matmul column block width
_NB = _B // _NBW          # 4 column blocks
_SCALE = 10.0             # 1/tau

_CACHE = {}


def _build_nc(reps=1):
    # reps>1 wraps the compute schedule in a hardware loop (timing builds
    # only; the body is idempotent so results are unchanged)
    import concourse.bass as bass
    import concourse.mybir as mybir
    from concourse import bacc, tile
    from contextlib import ExitStack

    f32 = mybir.dt.float32
    bf16 = mybir.dt.bfloat16
    fp8 = mybir.dt.float8e4
    DR = mybir.MatmulPerfMode.DoubleRow
    AF = mybir.ActivationFunctionType
    OP = mybir.AluOpType
    AX = mybir.AxisListType

    nc = bacc.Bacc("TRN2", target_bir_lowering=False, debug=False,
                   num_devices=_M)
    f8_d = nc.declare_dram_parameter("ft8", [_NB, _JT, 2, 128, _NBW], fp8, isOutput=False)
    l8_d = nc.declare_dram_parameter("fl8", [_JT, 2, 128, _BL], fp8, isOutput=False)
    fl_d = nc.declare_dram_parameter("ftloc", [_KT, 128, _BL], bf16, isOutput=False)
    rc_d = nc.declare_dram_parameter("rc", [_KT, 128, _C], bf16, isOutput=False)
    u_d = nc.declare_dram_parameter("u", [_KT, 128, _BL], bf16, isOutput=False)
    w_d = nc.declare_dram_parameter("wrow", [1, _B], f32, isOutput=False)
    v_d = nc.declare_dram_parameter("vrow", [1, _C], f32, isOutput=False)
    dg_d = nc.declare_dram_parameter("diagc", [128, _RT], f32, isOutput=False)
    rn_d = nc.declare_dram_parameter("rnp", [128, _RT], f32, isOutput=False)
    cr_d = nc.declare_dram_parameter("corrc", [128, _RT], f32, isOutput=False)
    id_d = nc.declare_dram_parameter("ident", [128, 128], f32, isOutput=False)
    on_d = nc.declare_dram_parameter("ones", [128, 1], f32, isOutput=False)
    out_d = nc.declare_dram_parameter("out", [1, 1], f32, isOutput=True)

    with tile.TileContext(nc) as tc, ExitStack() as ctx:
        consts = ctx.enter_context(tc.tile_pool(name="consts", bufs=1))
        psum = ctx.enter_context(tc.tile_pool(name="psum", bufs=1, space="PSUM"))
        sm = ctx.enter_context(tc.tile_pool(name="sm", bufs=8))

        # --- persistent SBUF residents -------------------------------------
        # HWDGE (sync) queue order = urgency: w/v rows (gate the first STTs
        # via the partition broadcasts), then the fp8 lhs/rhs chunks for the
        # first big block (j-granular, interleaved), then the rest merged.
        # Late-needed tensors ride the parallel gpsimd/SWDGE path.
        wrow = consts.tile([1, _B], f32, tag="wrow")
        nc.sync.dma_start(wrow[:], w_d[:])
        vrow = consts.tile([1, _C], f32, tag="vrow")
        nc.sync.dma_start(vrow[:], v_d[:])
        wb = consts.tile([128, _B], f32, tag="wb")
        nc.gpsimd.partition_broadcast(wb[:], wrow[:])
        vb = consts.tile([128, _C], f32, tag="vb")
        nc.gpsimd.partition_broadcast(vb[:], vrow[:])

        fl8 = consts.tile([128, _JT * 2 * _BL], fp8, tag="fl8")
        ft8 = [consts.tile([128, _JT * 2 * _NBW], fp8, tag=f"ft8_{n}", name=f"ft8_{n}")
               for n in range(_NB)]
        for j in range(_JT):
            nc.sync.dma_start(
                fl8[:, j * 2 * _BL:(j + 1) * 2 * _BL].rearrange(
                    "p (i c) -> p i c", i=2),
                l8_d[j].rearrange("i p c -> p i c"))
            nc.sync.dma_start(
                ft8[0][:, j * 2 * _NBW:(j + 1) * 2 * _NBW].rearrange(
                    "p (i c) -> p i c", i=2),
                f8_d[0, j].rearrange("i p c -> p i c"))
        for n in range(1, _NB):
            nc.sync.dma_start(
                ft8[n][:].rearrange("p (j i c) -> p j i c", j=_JT, i=2),
                f8_d[n].rearrange("j i p c -> p j i c"))

        fl = consts.tile([128, _KT * _BL], bf16, tag="fl")
        nc.sync.dma_start(fl[:].rearrange("p (k c) -> p k c", k=_KT),
                          fl_d[:].rearrange("k p c -> p k c"))
        dgc = consts.tile([128, _RT], f32, tag="dgc")
        nc.sync.dma_start(dgc[:], dg_d[:])
        rnp = consts.tile([128, _RT], f32, tag="rnp")
        nc.sync.dma_start(rnp[:], rn_d[:])
        corrc = consts.tile([128, _RT], f32, tag="corrc")
        nc.sync.dma_start(corrc[:], cr_d[:])

        rct = consts.tile([128, _KT * _C], bf16, tag="rct")
        nc.gpsimd.dma_start(rct[:].rearrange("p (k c) -> p k c", k=_KT),
                            rc_d[:].rearrange("k p c -> p k c"))
        ut = consts.tile([128, _KT * _BL], bf16, tag="ut")
        nc.gpsimd.dma_start(ut[:].rearrange("p (k c) -> p k c", k=_KT),
                            u_d[:].rearrange("k p c -> p k c"))
        ident = consts.tile([128, 128], f32, tag="ident")
        nc.gpsimd.dma_start(ident[:], id_d[:])
        ones = consts.tile([128, 1], f32, tag="ones")
        nc.gpsimd.dma_start(ones[:], on_d[:])

        # slice helpers
        lhs = [[fl[:, k * _BL + m * 128: k * _BL + (m + 1) * 128]
                for k in range(_KT)] for m in range(_RT)]
        lhs8 = [[fl8[:, j * 2 * _BL:(j + 1) * 2 * _BL]
                 .rearrange("p (i c) -> p i c", i=2)[:, :, m * 128:(m + 1) * 128]
                 for j in range(_JT)] for m in range(_RT)]
        accs = [consts.tile([128, 5], f32, tag=f"acc{m}", name=f"acc{m}")
                for m in range(_RT)]
        pstile = consts.tile([128, _RT], f32, tag="pstile")
        p4 = consts.tile([128, _RT], f32, tag="p4")
        denom4 = consts.tile([128, _RT], f32, tag="denom4")

        def big_block(n, m):
            ps = psum.tile([128, _NBW], f32, tag="big", bufs=3, name="psb")
            for j in range(_JT):
                rj = ft8[n][:, j * 2 * _NBW:(j + 1) * 2 * _NBW].rearrange(
                    "p (i c) -> p i c", i=2)
                for h in (0, 1):
                    nc.tensor.matmul(ps[:, h * 512:(h + 1) * 512], lhs8[m][j],
                                     rj[:, :, h * 512:(h + 1) * 512],
                                     start=(j == 0), stop=(j == _JT - 1),
                                     perf_mode=DR)
            nc.scalar.activation(ps[:], ps[:], AF.Exp, scale=_SCALE)
            nc.vector.scalar_tensor_tensor(
                out=ps[:], in0=ps[:], scalar=1.0,
                in1=wb[:, n * _NBW:(n + 1) * _NBW],
                op0=OP.mult, op1=OP.mult,
                accum_out=accs[m][:, n:n + 1])

        def centers_block(m):
            ps = psum.tile([128, _NBW], f32, tag="big", bufs=3, name="psc")
            for k in range(_KT):
                rk = rct[:, k * _C: (k + 1) * _C]
                nc.tensor.matmul(ps[:, 0:512], lhs[m][k], rk[:, 0:512],
                                 start=(k == 0), stop=(k == _KT - 1))
                nc.tensor.matmul(ps[:, 512:_C], lhs[m][k], rk[:, 512:_C],
                                 start=(k == 0), stop=(k == _KT - 1))
            nc.scalar.activation(ps[:, :_C], ps[:, :_C], AF.Exp, scale=_SCALE)
            nc.vector.scalar_tensor_tensor(
                out=ps[:, :_C], in0=ps[:, :_C], scalar=1.0, in1=vb[:],
                op0=OP.mult, op1=OP.mult,
                accum_out=accs[m][:, 4:5])

        def udiag_block(m):
            ps = psum.tile([128, _NBW], f32, tag="big", bufs=3, name="psu")
            for k in range(_KT):
                uk = ut[:, k * _BL + m * 128: k * _BL + (m + 1) * 128]
                nc.tensor.matmul(ps[:, :128], lhs[m][k], uk,
                                 start=(k == 0), stop=(k == _KT - 1))
            nc.vector.scalar_tensor_tensor(
                out=ps[:, :128], in0=ps[:, :128], scalar=1.0, in1=ident[:],
                op0=OP.mult, op1=OP.mult,
                accum_out=p4[:, m:m + 1])

        def finals():
            for m in range(_RT):
                nc.vector.tensor_reduce(denom4[:, m:m + 1], accs[m][:, 0:5],
                                        axis=AX.X, op=OP.add)
            # exact correction of the fp8 diagonal inside the denominator
            nc.vector.tensor_tensor(out=denom4[:], in0=denom4[:], in1=corrc[:],
                                    op=OP.add)
            logd = sm.tile([128, _RT], f32, tag="logd", name="logd")
            # denom is O(1e3); the reference's +1e-8 is far below fp32 ulp
            nc.scalar.activation(logd[:], denom4[:], AF.Ln)
            t1 = sm.tile([128, _RT], f32, tag="t1", name="t1")
            nc.vector.tensor_tensor(out=t1[:], in0=p4[:], in1=dgc[:], op=OP.subtract)
            nc.vector.tensor_tensor(out=t1[:], in0=t1[:], in1=rnp[:], op=OP.mult)
            nc.vector.tensor_tensor(out=pstile[:], in0=logd[:], in1=t1[:],
                                    op=OP.subtract)

        # --- main schedule --------------------------------------------------
        def body(_i=None):
            for m in range(_RT):
                big_block(0, m)
            for m in range(_RT):
                big_block(1, m)
            for m in range(_RT):
                centers_block(m)
                udiag_block(m)
            for m in range(_RT):
                big_block(2, m)
            for m in range(_RT):
                big_block(3, m)
            finals()

        if reps == 1:
            body()
        else:
            with tc.For_i(0, reps, 1) as i:
                body(i)

        # partition sum -> scalar partial (ones matmul reduces partitions)
        ps = psum.tile([128, _NBW], f32, tag="big", bufs=3, name="psf")
        nc.tensor.matmul(ps[:1, :_RT], ones[:], pstile[:], start=True, stop=True)
        final = consts.tile([1, 1], f32, tag="final")
        nc.vector.tensor_reduce(final[:], ps[:1, :_RT], axis=AX.X, op=OP.add)
        nc.sync.dma_start(out_d[:], final[:])

    nc.compile()
    return nc


def _get_nc():
    if "nc" not in _CACHE:
        _CACHE["nc"] = _build_nc()
    return _CACHE["nc"]


def _prep_inputs(centers, features, targets):
    bf16 = ml_dtypes.bfloat16
    fp8 = ml_dtypes.float8_e4m3
    F = np.ascontiguousarray(features, dtype=np.float32)      # [B, D]
    Cen = np.ascontiguousarray(centers, dtype=np.float32)     # [C, D]
    t = np.asarray(targets).astype(np.int64).ravel()          # [B]

    counts = np.bincount(t, minlength=_C).astype(np.float32)  # [C]
    w = (1.0 / (counts[t] + 1.0)).astype(np.float32)          # [B]
    v = (1.0 / (counts + 1.0)).astype(np.float32)             # [C]
    H = np.zeros((_C, _D), dtype=np.float32)
    np.add.at(H, t, F)                                        # class sums
    R2 = H + Cen                                              # [C, D]

    Fb = F.astype(bf16)                                       # bf16 features
    FT = np.ascontiguousarray(Fb.T)                           # [D, B] bf16
    F8 = F.astype(fp8)                                        # fp8 features
    FT8 = np.ascontiguousarray(F8.T)                          # [D, B] fp8
    # fp8 rhs chunks [n][j, i, p, c]: k = j*256 + i*128 + p
    ft8 = np.ascontiguousarray(
        FT8.reshape(_JT, 2, 128, _NB, _NBW).transpose(3, 0, 1, 2, 4))
    rc = np.ascontiguousarray(Cen.astype(bf16).T).reshape(_KT, 128, _C)
    U_all = R2.astype(bf16).T[:, t]                           # [D, B] gathered

    diag = (Fb.astype(np.float32) ** 2).sum(axis=1)           # matches bf16 paths
    diag8 = (F8.astype(np.float32) ** 2).sum(axis=1)          # fp8 device diag
    # denominator correction: replace exp(10*diag8)*w by exp(10*diag)*w
    corr = (w * (np.exp(np.float32(_SCALE) * diag)
                 - np.exp(np.float32(_SCALE) * diag8))).astype(np.float32)
    rnp = (np.float32(_SCALE) / counts[t]).astype(np.float32)

    wrow = w.reshape(1, _B)
    vrow = v.reshape(1, _C)
    ident = np.eye(128, dtype=np.float32)
    ones = np.ones((128, 1), dtype=np.float32)

    def col(x_loc):  # [512] -> [128, RT] with (p, m) = x[m*128+p]
        return np.ascontiguousarray(x_loc.reshape(_RT, 128).T)

    in_maps = []
    for c in range(_M):
        R = c * _BL
        ftloc = np.ascontiguousarray(FT[:, R:R + _BL]).reshape(_KT, 128, _BL)
        fl8 = np.ascontiguousarray(FT8[:, R:R + _BL]).reshape(_JT, 2, 128, _BL)
        uloc = np.ascontiguousarray(U_all[:, R:R + _BL]).reshape(_KT, 128, _BL)
        in_maps.append({
            "ft8": ft8, "fl8": fl8, "ftloc": ftloc, "rc": rc, "u": uloc,
            "wrow": wrow, "vrow": vrow,
            "diagc": col(diag[R:R + _BL]),
            "rnp": col(rnp[R:R + _BL]),
            "corrc": col(corr[R:R + _BL]),
            "ident": ident, "ones": ones,
        })
    return in_maps


def _run(inputs, trace=False, **trace_kwargs):
    from concourse.bass_utils import run_bass_kernel_spmd
    nc = _get_nc()
    in_maps = _prep_inputs(**inputs)
    res = run_bass_kernel_spmd(nc, in_maps, core_ids=list(range(_M)),
                               trace=trace, **trace_kwargs)
    total = sum(float(r["out"][0, 0]) for r in res.results)
    return np.float32(total / _B), res


def kernel(centers, features, targets):
    out, _ = _run({"centers": centers, "features": features, "targets": targets})
    return out
